# revision 8
# baseline (speedup 1.0000x reference)
"""AFT-Full attention kernel for 8 TRN2 NeuronCores.

Reference computation (S=2048, B=16, D=512):
    q = query @ Wq.T + bq
    k = key @ Wk.T + bk
    v = k @ Wv.T + bv
    num = exp_pb @ (exp(k) * v);  den = exp_pb @ exp(k)   (per batch)
    out = (sigmoid(q) * num / den).transpose(1,0,2) @ Wo.T + bo

Sharding: data-parallel over batch B: 2 batches per core, no collectives.

Math notes:
  - the max-subtractions in the reference cancel exactly in num/den.
  - v = k @ Wv.T = key @ (Wv @ Wk).T (host-folded weight).
  - bq/bk absorbed into query/key on the host; bo added on the host.
  - exp_pb = 1 + R with R = expm1(pos_bias), |R| ~ 0.02.  The rank-1
    ones part becomes a column-sum of exp(k) / exp(k)*v (computed during
    phase 1 and reduced across partitions with tiny matmuls); the
    residual einsum R @ X runs in fp8 (e4m3) with the DoubleRow perf
    mode: contraction 256 deep per instruction, 2x the bf16 matmul
    throughput.  Quantization error lands only on the ~2%-magnitude
    residual, so the result is *more* accurate than a bf16 einsum.
    Scales keep every fp8 value under 240 (e4m3-safe):
        R8 = R * 2^9,  E8 = exp(k) * 2^-3,  Ev8 = exp(k)*v * 2^-4
    and the gate de-scales with (nd * 2^-6 + csE) / (nd * 2^-5 + csX).
  - the q projection also runs in fp8 DoubleRow (Wq scaled by 2^6 on
    the host, de-scaled inside the sigmoid): q only feeds sigmoid(q),
    whose 0.25 Lipschitz bound keeps the fp8 error ~1% of the output.
  - matmuls accumulate in chains over a fixed PSUM bank.

The einsum computes num/den TRANSPOSED (numT[d, i]), so gating and the
output projection run in [d, s] layout with no PE transposes.  The
gating of chunk T is emitted after the einsum of chunk T+1 so the
TensorEngine never idles waiting for the vector-engine epilogue.

All input DMAs are issued up front in consumption-priority order and
phase-2 operands (qT, wq, wo) live in persistent SBUF, so their loads
overlap phase-1 compute instead of stalling the phase transition.
"""
import sys

sys.path.insert(0, "/opt/trn_rl_repo")

import numpy as np

S, B, D = 2048, 16, 512
NCORES = 8
BLOC = B // NCORES          # 2 batches per core
ST = S // 128               # 16 seq (j) tiles
DT = D // 128               # 4 feature tiles
NC = S // 512               # 4 output column-chunks (512 wide)

LN8 = float(np.log(8.0))    # E8 = exp(k - 3ln2) = exp(k)/8
QSCALE = 64.0               # Wq host-scaled by 2^6 for the fp8 q-proj

_cache = {}


def _build(use_kv: bool):
    import concourse.bacc as bacc
    import concourse.mybir as mybir
    import concourse.tile as tile

    f32 = mybir.dt.float32
    bf16 = mybir.dt.bfloat16
    fp8 = mybir.dt.float8e4
    ACT = mybir.ActivationFunctionType
    ALU = mybir.AluOpType
    DR = mybir.MatmulPerfMode.DoubleRow

    nc = bacc.Bacc()

    # key pre-tiled partition-major: [b, p, st, kt, 128] (lhsT tiles for the
    # k/v projections); element = key.T[kt*128+p, st*128+sl] per batch
    kT = nc.declare_dram_parameter("kT", [BLOC, 128, ST, DT, 128], bf16, isOutput=False)
    kTv = (
        nc.declare_dram_parameter("kTv", [BLOC, 128, ST, DT, 128], bf16, isOutput=False)
        if use_kv
        else kT
    )
    # query pre-tiled as moving tiles, fp8: [b, p, kt, s]; element = q.T[kt*128+p, s]
    qT8 = nc.declare_dram_parameter("qT8", [BLOC, 128, DT, S], fp8, isOutput=False)
    # R8 = expm1(pos_bias).T * 512, tiled [p, jt, i]; fp8 e4m3
    pbt8 = nc.declare_dram_parameter("pbt8", [128, ST, S], fp8, isOutput=False)
    # weights pre-tiled: [p, kt, dout] with din = kt*128+p
    wk = nc.declare_dram_parameter("wk", [128, DT, D], bf16, isOutput=False)
    wvk = nc.declare_dram_parameter("wvk", [128, DT, D], bf16, isOutput=False)
    wq8 = nc.declare_dram_parameter("wq8", [128, DT, D], fp8, isOutput=False)
    wo = nc.declare_dram_parameter("wo", [128, DT, D], bf16, isOutput=False)
    out = nc.declare_dram_parameter("out", [BLOC, S, D], f32, isOutput=True)

    with tile.TileContext(nc) as tc:
        with (
            tc.tile_pool(name="big", bufs=1) as big,
            tc.tile_pool(name="psum", bufs=1, space="PSUM") as psum,
        ):
            # persistent fp8 exp(k)/8, exp(k)*v/16 per local batch: [p, jt, d]
            E8 = [big.tile([128, ST, D], fp8, name=f"E8{b}") for b in range(BLOC)]
            Ev8 = [big.tile([128, ST, D], fp8, name=f"Ev8{b}") for b in range(BLOC)]
            # f32 per-partition running sum of exp(k)*v over jt (rank-1 part
            # of num); den is pure rank-1 and comes from E8 column sums
            sumEv = [big.tile([128, D], f32, name=f"sumEv{b}") for b in range(BLOC)]
            # whole residual matrix R8, resident: 4 MB fp8
            R8 = big.tile([128, ST, S], fp8, name="R8")
            # phase-2 operands, persistent so their DMAs overlap phase 1
            qfull = [big.tile([128, DT, S], fp8, name=f"qfull{b}") for b in range(BLOC)]
            wq_sb = big.tile([128, DT, D], fp8, name="wq_sb")
            wo_sb = big.tile([128, DT, D], bf16, name="wo_sb")
            # gate constants: w = ps_num*C1 + C2 with C1 = 2^-5/csE,
            # C2 = csX/csE  (den ~= csE: its residual is ~0.07% and dropped)
            C1 = [big.tile([128, DT], f32, name=f"C1_{b}") for b in range(BLOC)]
            C2 = [big.tile([128, DT], f32, name=f"C2_{b}") for b in range(BLOC)]
            warm_src = big.tile([128, 128], bf16, name="warm_src")
            nc.vector.memset(warm_src[:, :], 1.0)
            ones_col = big.tile([128, 1], f32, name="ones_col")
            nc.vector.memset(ones_col[:, :], 1.0)
            # DoubleRow all-ones stationary: [Ki, Ko=2, 1] with 16-aligned
            # Ko stride (the 16-wide last dim exists only for the stride)
            ones8dr = big.tile([128, 2, 16], fp8, name="ones8dr")
            nc.vector.memset(ones8dr[:, :, :], 1.0)
            bias8 = big.tile([128, 1], f32, name="bias8")
            nc.vector.memset(bias8[:, :], -LN8)
            for b in range(BLOC):
                nc.gpsimd.memset(sumEv[b][:, :], 0.0)

            # PE warmup: keep TensorE busy while the first DMAs stream so the
            # HAM clock-gate opens before the first real matmul
            ps_warm = psum.tile([128, 128], f32, tag="ps_q", bufs=2)
            for _ in range(26):
                nc.tensor.matmul(ps_warm[:, :], warm_src[:, :], warm_src[:, :])

            # ---------------- phase 1: projections k, v -> E8, Ev8 ----------
            with (
                tc.tile_pool(name="ph1", bufs=1) as ph1,
                tc.tile_pool(name="scr", bufs=3) as scr,
            ):
                wk_sb = ph1.tile([128, DT, D], bf16)
                wvk_sb = ph1.tile([128, DT, D], bf16)
                kfull = [
                    ph1.tile([128, ST, DT, 128], bf16, name=f"kfull{b}")
                    for b in range(BLOC)
                ]
                if use_kv:
                    kvfull = [
                        ph1.tile([128, ST, DT, 128], bf16, name=f"kvfull{b}")
                        for b in range(BLOC)
                    ]
                else:
                    kvfull = kfull

                # all input DMAs up front, in consumption-priority order;
                # graduated chunks so batch 0's first j-tiles land early
                CHUNKS = [(0, 1), (1, 2), (2, 4), (4, 8), (8, 16)]
                for kt in range(DT):
                    nc.sync.dma_start(wk_sb[:, kt], wk[:, kt])
                for lo, hi in CHUNKS[:3]:
                    nc.sync.dma_start(kfull[0][:, lo:hi], kT[0, :, lo:hi])
                for kt in range(DT):
                    nc.sync.dma_start(wvk_sb[:, kt], wvk[:, kt])
                for lo, hi in CHUNKS[3:]:
                    nc.sync.dma_start(kfull[0][:, lo:hi], kT[0, :, lo:hi])
                if use_kv:
                    for lo, hi in CHUNKS:
                        nc.sync.dma_start(kvfull[0][:, lo:hi], kTv[0, :, lo:hi])
                for lo, hi in CHUNKS:
                    nc.sync.dma_start(kfull[1][:, lo:hi], kT[1, :, lo:hi])
                if use_kv:
                    for lo, hi in CHUNKS:
                        nc.sync.dma_start(kvfull[1][:, lo:hi], kTv[1, :, lo:hi])
                for c in range(4):
                    csl = slice(c * (S // 4), (c + 1) * (S // 4))
                    nc.sync.dma_start(R8[:, :, csl], pbt8[:, :, csl])
                for b in range(BLOC):
                    for c in range(4):
                        csl = slice(c * (S // 4), (c + 1) * (S // 4))
                        nc.sync.dma_start(qfull[b][:, :, csl], qT8[b, :, :, csl])
                nc.sync.dma_start(wq_sb[:, :, :], wq8[:, :, :])
                nc.sync.dma_start(wo_sb[:, :, :], wo[:, :, :])

                for b in range(BLOC):
                    for jt in range(ST):
                        # alternate PSUM tags by jt parity so the next chain
                        # can accumulate while this one's epilogue drains
                        ps_k = psum.tile(
                            [128, D], f32, tag=f"nd{jt % 2 * 2}", name=f"ps_k{jt}"
                        )
                        for kt in range(DT):
                            nc.tensor.matmul(
                                ps_k[:, :],
                                kfull[b][:, jt, kt, :],
                                wk_sb[:, kt, :],
                                start=(kt == 0),
                                stop=(kt == DT - 1),
                            )
                        ps_v = psum.tile(
                            [128, D], f32, tag=f"nd{jt % 2 * 2 + 1}", name=f"ps_v{jt}"
                        )
                        for kt in range(DT):
                            nc.tensor.matmul(
                                ps_v[:, :],
                                kvfull[b][:, jt, kt, :],
                                wvk_sb[:, kt, :],
                                start=(kt == 0),
                                stop=(kt == DT - 1),
                            )
                        E_scr = scr.tile([128, D], f32, tag="E_scr")
                        nc.scalar.activation(E_scr[:, :], ps_k[:, :], ACT.Exp)
                        nc.scalar.activation(
                            E8[b][:, jt, :], ps_k[:, :], ACT.Exp, bias=bias8[:, :]
                        )
                        Ev_scr = scr.tile([128, D], f32, tag="Ev_scr")
                        nc.vector.tensor_mul(Ev_scr[:, :], E_scr[:, :], ps_v[:, :])
                        # fp8 cast must stay on DVE (GpSimd converts fp8 in
                        # software, ~9us); the in-place accumulate must stay
                        # OFF DVE (DVE in-place hits an element-serial path)
                        nc.vector.tensor_scalar(
                            Ev8[b][:, jt, :], Ev_scr[:, :], 0.0625, None, ALU.mult
                        )
                        nc.gpsimd.tensor_add(
                            sumEv[b][:, :], sumEv[b][:, :], Ev_scr[:, :]
                        )

            # ------- phase 2: transposed einsum + gating + output -----------
            with (
                tc.tile_pool(name="fin", bufs=2) as fin,
                tc.tile_pool(name="scr2", bufs=1) as scr2,
                tc.tile_pool(name="yts", bufs=2) as yts,
            ):
                rowE = [scr2.tile([1, S // 4], f32, name=f"rowE{b}") for b in range(BLOC)]
                rowX = [scr2.tile([1, S // 4], f32, name=f"rowX{b}") for b in range(BLOC)]

                def cs_sums():
                    # Rank-1 column sums as PSUM rows:
                    #   csE/8 = sum_j E8[j, d]   (fp8 DoubleRow chain, [1, 512])
                    #   csX   = sum_j Ev[j, d]   (f32r matmul on sumEv)
                    # copied to SBUF rows on scalar/vector for cs_finish.
                    for b in range(BLOC):
                        ps_rE = psum.tile(
                            [1, S // 4], f32, tag="ps_q", bufs=2, name=f"ps_rE{b}"
                        )
                        for jp in range(ST // 2):
                            pr = slice(2 * jp, 2 * jp + 2)
                            nc.tensor.matmul(
                                ps_rE[:, :],
                                ones8dr[:, :, 0:1],
                                E8[b][:, pr, :],
                                start=(jp == 0),
                                stop=(jp == ST // 2 - 1),
                                perf_mode=DR,
                            )
                        ps_rX = psum.tile(
                            [1, S // 4], f32, tag="ps_o", bufs=2, name=f"ps_rX{b}"
                        )
                        nc.tensor.matmul(
                            ps_rX[:, :],
                            ones_col[:, :],
                            sumEv[b][:, :],
                            start=True,
                            stop=True,
                        )
                        nc.scalar.copy(rowE[b][:, :], ps_rE[:, :])
                        nc.vector.tensor_scalar(
                            rowX[b][:, :], ps_rX[:, :], 1.0, None, ALU.mult
                        )

                def cs_finish():
                    # transpose the rows onto the d-partitions with tiny
                    # matmuls, then all the divides run on [128, DT] tiles
                    # (vector-parallel) instead of element-serial rows.
                    for b in range(BLOC):
                        ps_cE = psum.tile(
                            [128, DT], f32, tag="ps_q", bufs=2, name=f"ps_cE{b}"
                        )
                        for m in range(DT):
                            nc.tensor.matmul(
                                ps_cE[:, m : m + 1],
                                rowE[b][:, m * 128 : (m + 1) * 128],
                                ones_col[0:1, :],
                                start=True,
                                stop=True,
                            )
                        ps_cX = psum.tile(
                            [128, DT], f32, tag="ps_o", bufs=2, name=f"ps_cX{b}"
                        )
                        for m in range(DT):
                            nc.tensor.matmul(
                                ps_cX[:, m : m + 1],
                                rowX[b][:, m * 128 : (m + 1) * 128],
                                ones_col[0:1, :],
                                start=True,
                                stop=True,
                            )
                        rec = scr2.tile([128, DT], f32, tag="rec")
                        nc.vector.reciprocal(rec[:, :], ps_cE[:, :])
                        # C1 = 2^-8 * (8/csE) = 2^-5/csE
                        nc.vector.tensor_scalar(
                            C1[b][:, :], rec[:, :], 0.00390625, None, ALU.mult
                        )
                        c2t = scr2.tile([128, DT], f32, tag="c2t")
                        nc.vector.tensor_mul(c2t[:, :], ps_cX[:, :], rec[:, :])
                        nc.vector.tensor_scalar(
                            C2[b][:, :], c2t[:, :], 0.125, None, ALU.mult
                        )

                def einsum_step(n, m):
                    # numT [d-chunk 128, i-chunk 512] residual for both
                    # batches; fp8 DoubleRow, 8-deep chains on a fixed PSUM
                    # bank.  Results stay in PSUM; the gate reads them there.
                    nsl = slice(n * 512, (n + 1) * 512)
                    msl = slice(m * 128, (m + 1) * 128)
                    par = (n * DT + m) % 2
                    ps_nd = []
                    for b in range(BLOC):
                        ps = psum.tile(
                            [128, 512], f32, tag=f"nd{2 * par + b}",
                            name=f"nd{b}_{n}_{m}",
                        )
                        for jp in range(ST // 2):
                            pr = slice(2 * jp, 2 * jp + 2)
                            nc.tensor.matmul(
                                ps[:, :],
                                Ev8[b][:, pr, msl],
                                R8[:, pr, nsl],
                                start=(jp == 0),
                                stop=(jp == ST // 2 - 1),
                                perf_mode=DR,
                            )
                        ps_nd.append(ps)
                    return ps_nd

                def gate_step(n, m, ps_nd, yT):
                    # fp8 DoubleRow qT chunk, sigmoid (with the 2^-6 Wq
                    # de-scale), fused de-scale + rank-1 + divide
                    # -> yT[b][:, m, :] in [d, s] layout
                    nsl = slice(n * 512, (n + 1) * 512)
                    msl = slice(m * 128, (m + 1) * 128)
                    for b in range(BLOC):
                        ps_q = psum.tile([128, 512], f32, tag="ps_q", bufs=2)
                        for t in range(DT // 2):
                            tp = slice(2 * t, 2 * t + 2)
                            nc.tensor.matmul(
                                ps_q[:, :],
                                wq_sb[:, tp, msl],
                                qfull[b][:, tp, nsl],
                                start=(t == 0),
                                stop=(t == DT // 2 - 1),
                                perf_mode=DR,
                            )
                        sig = fin.tile([128, 512], f32, tag="sig")
                        nc.scalar.activation(
                            sig[:, :], ps_q[:, :], ACT.Sigmoid, scale=1.0 / QSCALE
                        )
                        w = fin.tile([128, 512], f32, tag="w")
                        nc.vector.tensor_scalar(
                            w[:, :],
                            ps_nd[b][:, :],
                            C1[b][:, m : m + 1],
                            C2[b][:, m : m + 1],
                            ALU.mult,
                            ALU.add,
                        )
                        nc.gpsimd.tensor_mul(yT[b][:, m, :], w[:, :], sig[:, :])

                def output_step(n, yT):
                    # out[s, dout] for the 4 s-subtiles of this n-chunk;
                    # PSUM->SBUF copies alternate scalar/vector so the final
                    # drain isn't serialized on one engine
                    for b in range(BLOC):
                        for ssub in range(4):
                            s0 = n * 512 + ssub * 128
                            ps_o = psum.tile([128, D], f32, tag="ps_o", bufs=2)
                            for dk in range(DT):
                                nc.tensor.matmul(
                                    ps_o[:, :],
                                    yT[b][:, dk, ssub * 128 : (ssub + 1) * 128],
                                    wo_sb[:, dk, :],
                                    start=(dk == 0),
                                    stop=(dk == DT - 1),
                                )
                            o_sb = fin.tile([128, D], f32, tag="o_sb")
                            if ssub % 2 == 0:
                                nc.scalar.copy(o_sb[:, :], ps_o[:, :])
                            else:
                                nc.vector.tensor_scalar(
                                    o_sb[:, :], ps_o[:, :], 1.0, None, ALU.mult
                                )
                            nc.sync.dma_start(out[b, s0 : s0 + 128, :], o_sb[:, :])

                # software pipeline over (n, m) chunks: gate(prev) after
                # einsum(cur), and each n-chunk's output projection deferred
                # one further einsum so its dk=3 matmul never waits on the
                # just-emitted gate's gpsimd epilogue
                cs_sums()
                prev = None          # (n, m, nd_sb)
                pending_out = None
                yT_tiles = {}
                for n in range(NC):
                    yT_tiles[n] = [
                        yts.tile([128, DT, 512], bf16, tag=f"yT{b}", name=f"yT{b}_{n}")
                        for b in range(BLOC)
                    ]
                    for m in range(DT):
                        nd_sb = einsum_step(n, m)
                        if n == 0 and m == 0:
                            cs_finish()
                        flush, pending_out = pending_out, None
                        if prev is not None:
                            pn, pm, pnd = prev
                            gate_step(pn, pm, pnd, yT_tiles[pn])
                            if pm == DT - 1:
                                pending_out = pn
                        if flush is not None:
                            output_step(flush, yT_tiles[flush])
                            del yT_tiles[flush]
                        prev = (n, m, nd_sb)
                pn, pm, pnd = prev
                gate_step(pn, pm, pnd, yT_tiles[pn])
                output_step(pn, yT_tiles[pn])

    nc.compile()
    return nc


def _tile_act(xT):
    """[D, S] -> [p, st, kt, 128] partition-major host tiling (lhsT tiles)."""
    z = xT.reshape(DT, 128, ST, 128)
    return np.ascontiguousarray(z.transpose(1, 2, 0, 3))


def _tile_mov(xT):
    """[D, S] -> [p, kt, S] partition-major host tiling (moving tiles)."""
    z = xT.reshape(DT, 128, S)
    return np.ascontiguousarray(z.transpose(1, 0, 2))


def _tile_w(wT):
    """[D, D] (din, dout) -> [p, kt, dout] with din = kt*128+p."""
    return np.ascontiguousarray(wT.reshape(DT, 128, D).transpose(1, 0, 2))


def _prep(query, key, Wq, bq, Wk, bk, Wv, bv, pos_bias, Wo, bo):
    """Host-side preprocessing: transposes, tiling, bias absorption, bf16."""
    import ml_dtypes

    bf16 = ml_dtypes.bfloat16
    e4m3 = ml_dtypes.float8_e4m3

    query = np.asarray(query, dtype=np.float32)
    key = np.asarray(key, dtype=np.float32)
    Wq = np.asarray(Wq, dtype=np.float32)
    Wk = np.asarray(Wk, dtype=np.float32)
    Wv = np.asarray(Wv, dtype=np.float32)
    Wo = np.asarray(Wo, dtype=np.float32)
    bq = np.asarray(bq, dtype=np.float32)
    bk = np.asarray(bk, dtype=np.float32)
    bv = np.asarray(bv, dtype=np.float32)
    bo = np.asarray(bo, dtype=np.float32)

    Wvk = Wv @ Wk

    if np.any(bq):
        query = query + np.linalg.solve(Wq, bq).astype(np.float32)
    if np.any(bk):
        key_k = key + np.linalg.solve(Wk, bk).astype(np.float32)
    else:
        key_k = key
    use_kv = bool(np.any(bv)) or bool(np.any(bk))
    if use_kv:
        bv_eff = Wv @ bk + bv
        key_v = key + np.linalg.solve(Wvk, bv_eff).astype(np.float32)
    else:
        key_v = None

    # [S, B, D] -> per-batch [D, S] -> tiled; query goes to fp8 e4m3
    qTb = query.transpose(1, 2, 0).astype(e4m3)
    kTb = key_k.transpose(1, 2, 0).astype(bf16)
    qT8 = np.stack([_tile_mov(qTb[b]) for b in range(B)])
    kT = np.stack([_tile_act(kTb[b]) for b in range(B)])
    if use_kv:
        kvb = key_v.transpose(1, 2, 0).astype(bf16)
        kTv = np.stack([_tile_act(kvb[b]) for b in range(B)])
    else:
        kTv = None

    # R8 = expm1(pos_bias).T * 512 tiled [p, jt, i], fp8 e4m3
    RT = np.expm1(np.asarray(pos_bias, dtype=np.float32)).T * 512.0
    pbt8 = np.ascontiguousarray(
        RT.reshape(ST, 128, S).transpose(1, 0, 2)
    ).astype(e4m3)

    wk = _tile_w(np.ascontiguousarray(Wk.T).astype(bf16))
    wvk = _tile_w(np.ascontiguousarray(Wvk.T).astype(bf16))
    wq8 = _tile_w(np.ascontiguousarray(Wq.T * QSCALE)).astype(e4m3)
    wo = _tile_w(np.ascontiguousarray(Wo.T).astype(bf16))
    return qT8, kT, kTv, pbt8, wk, wvk, wq8, wo, bo, use_kv


def kernel(query, key, Wq, bq, Wk, bk, Wv, bv, pos_bias, Wo, bo):
    from concourse.bass_utils import run_bass_kernel_spmd

    qT8, kT, kTv, pbt8, wk, wvk, wq8, wo, bo, use_kv = _prep(
        query, key, Wq, bq, Wk, bk, Wv, bv, pos_bias, Wo, bo
    )

    if ("nc", use_kv) not in _cache:
        _cache[("nc", use_kv)] = _build(use_kv)
    nc = _cache[("nc", use_kv)]

    in_maps = []
    for c in range(NCORES):
        bsl = slice(c * BLOC, (c + 1) * BLOC)
        m = {
            "qT8": qT8[bsl],
            "kT": kT[bsl],
            "pbt8": pbt8,
            "wk": wk,
            "wvk": wvk,
            "wq8": wq8,
            "wo": wo,
        }
        if use_kv:
            m["kTv"] = kTv[bsl]
        in_maps.append(m)

    res = run_bass_kernel_spmd(nc, in_maps, core_ids=list(range(NCORES)))
    out = np.concatenate([res.results[c]["out"] for c in range(NCORES)], axis=0)
    if np.any(bo):
        out = out + bo
    return out
